# revision 1
# baseline (speedup 1.0000x reference)
"""Trainium2 Bass kernel for a binarized-conv BasicBlock (2x BinConv3x3 + BN + residual + PReLU).

Strategy (8 NeuronCores, data-parallel over batch):
  - 64 images -> 8 per core; binarized conv weights / BN / PReLU params replicated.
  - Binarized values are exactly +/-1, so fp8 matmuls are numerically exact
    (fp32 PSUM accumulation of small integers). perf_mode=DoubleRow packs the
    full 256-channel contraction into one matmul per 3x3 tap.
  - Conv3x3 as implicit GEMM: per output tile [128 Cout x 392 cols] accumulate
    9 tap matmuls reading shifted windows of a zero-padded (30x30) binarized
    activation image.
  - BatchNorm uses full-batch statistics: per-channel (mean, E[y^2]) partials
    via bn_stats fused with PSUM evacuation, one tiny AllGather per BN stage.
    Two warm-up collectives issued first absorb the ~40us first-collective
    barrier under conv1.
  - Post-BN chain is fused: affine_then_add does A*y + B + residual in one DVE
    op (A = 8*gamma*rsqrt(u + 64*eps/s^2) with u = 8*Sum(E[y^2]) - Sum(mean)^2,
    all per-channel constants precomputed on the host); sign() for the next
    conv binarization runs straight on that sum (sign(prelu(t)) == sign(t)),
    so PReLU is off the critical path and is deferred into the conv2 window.
  - Image-0's post chain is processed in two row-chunks so conv2 can start
    after only ~half the post latency; remaining images pipeline under conv2.
  - Tail (post-BN2) is engine-balanced: DVE affine_then_add for images 0-4,
    GpSimd scalar_tensor_tensor for 5-7, PReLU split ACT/DVE, per-image
    output DMA overlap.
"""

import numpy as np
import ml_dtypes

import concourse.bacc as bacc
import concourse.mybir as mybir
import concourse.tile as tile
from concourse.tile_rust import add_dep_helper
from concourse import bass_utils

N_CORES = 8
B_FULL, C, H, W = 64, 256, 28, 28
BL = B_FULL // N_CORES  # images per core
P = 128
NB = C // P             # channel blocks
HW = H * W              # 784
PADL = 30               # padded row length
PADQ = PADL * PADL      # 900 padded image
HALF = 14 * W           # 392 columns per psum tile (half an image)
SCALE = 0.1
BN_EPS = 1e-5

F32 = mybir.dt.float32
BF16 = mybir.dt.bfloat16
FP8 = mybir.dt.float8e4
BF16_NP = np.dtype(ml_dtypes.bfloat16)

_CACHE: dict = {}

# par slots: 0:G8_1  1:E64_1  2:beta1  3:G8_2  4:E64_2  5:beta2  6:(a1,a2)
NSLOT = 7


def _build():
    nc = bacc.Bacc("TRN2", target_bir_lowering=False, debug=False,
                   num_devices=N_CORES)
    F = mybir.ActivationFunctionType
    Op = mybir.AluOpType
    DR = mybir.MatmulPerfMode.DoubleRow

    x_d = nc.dram_tensor("x", [BL, C, H, W], F32, kind="ExternalInput")
    xh_d = nc.dram_tensor("xh", [BL, C, H, W], BF16, kind="ExternalInput")
    # weights packed [ki, tap, i, mblk, co] with channel c = i*128 + ki
    w1_d = nc.dram_tensor("w1", [P, 9, NB, NB, P], FP8, kind="ExternalInput")
    w2_d = nc.dram_tensor("w2", [P, 9, NB, NB, P], FP8, kind="ExternalInput")
    par_d = nc.dram_tensor("par", [P, NSLOT, NB], F32, kind="ExternalInput")
    o_d = nc.dram_tensor("o", [BL, C, H, W], BF16, kind="ExternalOutput")

    with tile.TileContext(nc) as tc:
        with (
            tc.tile_pool(name="sbuf", bufs=1) as sbuf,
            tc.tile_pool(name="psum", bufs=8, space="PSUM") as psum_pool,
            tc.tile_pool(name="dram", bufs=1, space="DRAM") as dram,
        ):
            # ---- static tiles ----
            w1_sb = sbuf.tile([P, 9, NB, NB, P], FP8)
            w2_sb = sbuf.tile([P, 9, NB, NB, P], FP8)
            par_sb = sbuf.tile([P, NSLOT, NB], F32)
            g8_1, e64_1, be1 = (par_sb[:, i, :] for i in range(3))
            g8_2, e64_2, be2 = (par_sb[:, i, :] for i in range(3, 6))
            a1_ap = par_sb[:, 6, 0:1]
            a2_ap = par_sb[:, 6, 1:2]

            # xh holds the bf16 input copy; after binarization it is dead and
            # is reused as the bf16 conv2-output (y2) evacuation target
            xh_sb = sbuf.tile([P, BL, NB, HW], BF16)
            x_sb = sbuf.tile([P, BL, NB, HW], F32)
            y_sb = sbuf.tile([P, BL, NB, HW], F32)
            r2_sb = sbuf.tile([P, BL, NB, HW], BF16)  # prelu1 out (stage-2 res)
            xb_sb = sbuf.tile([P, BL, NB, PADQ], FP8)
            xbv = [xb_sb[:, b].rearrange("p k (r c) -> p k r c", c=PADL)
                   for b in range(BL)]
            xhv = [xh_sb[:, b].rearrange("p k (r c) -> p k r c", c=W)
                   for b in range(BL)]
            st6_1 = sbuf.tile([P, NB, BL * 12], F32)
            st6_2 = sbuf.tile([P, NB, BL * 12], F32)

            # ---- warm-up collectives: issued first so the ~40us
            # first-collective barrier burns off under conv1 ----
            warm_in = dram.tile([4, 2], F32, name="warm_in")
            warm_out = dram.tile([N_CORES * 4, 2], F32, name="warm_out",
                                 addr_space="Shared")
            warm_out2 = dram.tile([N_CORES * 4, 2], F32, name="warm_out2",
                                  addr_space="Shared")
            # ---- input DMAs: spread across queues (each DMA carries ~2us
            # fixed descriptor latency and serializes within its queue).
            # Image-0 chunks and the first weight half land in parallel. ----
            xh_src = xh_d.rearrange("b (k p) h w -> b p k (h w)", p=P)
            CH0 = 15 * W  # first row-chunk: image rows 0..14
            nc.sync.dma_start(w1_sb[:, 0:5], w1_d[:, 0:5])
            nc.scalar.dma_start(xh_sb[:, 0, :, 0:CH0], xh_src[0, :, :, 0:CH0])
            nc.gpsimd.dma_start(xh_sb[:, 0, :, CH0:HW],
                                xh_src[0, :, :, CH0:HW])
            nc.sync.dma_start(w1_sb[:, 5:9], w1_d[:, 5:9])
            nc.gpsimd.dma_start(par_sb[:], par_d[:, :, :])
            # ACT-table pre-warm: load the SIGN table before image-0's data
            # lands (the sqrt/prelu table set loads later, under conv1)
            tw = sbuf.tile([P, 2], F32, name="tblwarm")
            nc.vector.memset(tw[:], 1.0)
            nc.scalar.sign(tw[:, 0:1], tw[:, 1:2])
            last_xh_dma = None
            for b in range(1, BL):
                last_xh_dma = nc.sync.dma_start(xh_sb[:, b], xh_src[b])

            # pad-border memsets (DVE, cheap, image 0 first)
            for b in range(BL):
                nc.vector.memset(xbv[b][:, :, 0:30:29, :], 0.0)
                nc.vector.memset(xbv[b][:, :, 1:29, 0:30:29], 0.0)

            # residual x (f32) + conv2 weights stream after the bf16 x
            x_src = x_d.rearrange("b (k p) h w -> b p k (h w)", p=P)
            for b in range(BL):
                dma = nc.gpsimd.dma_start(x_sb[:, b], x_src[b])
                add_dep_helper(dma.ins, last_xh_dma.ins, sync=True,
                               reason="x f32 after bf16 x")
                if b == 1:
                    w2dma = nc.gpsimd.dma_start(w2_sb[:], w2_d[:, :, :, :, :])
                    add_dep_helper(w2dma.ins, last_xh_dma.ins, sync=True,
                                   reason="w2 after bf16 x")

            def head_sign(b):
                if b == 0:  # row-chunked so conv1 can start after 15 rows
                    nc.scalar.sign(xbv[0][:, :, 1:16, 1:29],
                                   xhv[0][:, :, 0:15, :])
                    nc.scalar.sign(xbv[0][:, :, 16:29, 1:29],
                                   xhv[0][:, :, 15:28, :])
                else:
                    nc.scalar.sign(xbv[b][:, :, 1:29, 1:29],
                                   xhv[b][:, :, :, :])

            def conv_img(w_sb, y_dst, st6, b):
                """one image's bin-conv3x3: 2 halves x 2 cout blocks x 9 tap
                matmuls; ACT evacuates PSUM, DVE takes bn stats."""
                for hh in range(2):
                    for m in range(NB):
                        ps = psum_pool.tile([P, HALF], F32, name="ps",
                                            tag="ps")
                        for t in range(9):
                            dh, dw = t // 3, t % 3
                            rhs = xbv[b][:, :,
                                         hh * 14 + dh:hh * 14 + dh + 14,
                                         dw:dw + 28]
                            nc.tensor.matmul(
                                ps[:], w_sb[:, t, :, m, :], rhs,
                                start=(t == 0), stop=(t == 8),
                                perf_mode=DR)
                        nc.scalar.copy(y_dst[:, m, hh * HALF:(hh + 1) * HALF],
                                       ps[:])
                        idx = (b * 2 + hh) * 6
                        nc.vector.bn_stats(st6[:, m, idx:idx + 6], ps[:])

            # ================= conv1 (head signs pipelined in) =============
            head_sign(0)  # row-chunked automatically via DMA split deps
            head_sign(1)
            # load the sqrt/prelu ACT-table set now (hidden under conv1)
            nc.scalar.sqrt(tw[:, 0:1], tw[:, 1:2])
            nc.scalar.activation(tw[:, 0:1], tw[:, 1:2], F.Prelu,
                                 bias=0.0, scale=1.0, alpha=0.25)
            for b in range(BL):
                if b + 2 < BL:
                    head_sign(b + 2)
                conv_img(w1_sb, y_sb[:, b], st6_1, b)

            # ================= BN stats -> affine params ===================
            def stats_to_ab(st6, g8_ap, e64_ap, be_ap, tagn):
                st2 = sbuf.tile([P, NB, 2], F32, name=f"st2_{tagn}")
                for m in range(NB):
                    nc.vector.bn_aggr(st2[:, m], st6[:, m])
                # local E[y^2] = var + mean^2 before the collective, so the
                # CCE sums (mean, E[y^2]) across ranks inside the AllReduce
                # and the 5-op post-gather conversion+tree disappears.
                tmm = sbuf.tile([P, NB], F32, name=f"tmm_{tagn}")
                nc.vector.tensor_tensor(tmm[:], st2[:, :, 0],
                                        st2[:, :, 0], Op.mult)
                nc.vector.tensor_tensor(st2[:, :, 1], st2[:, :, 1],
                                        tmm[:], Op.add)
                cc_din = dram.tile([P, 4], F32, name=f"ccdin_{tagn}")
                cc_dout = dram.tile([P, 4], F32, name=f"ccdout_{tagn}",
                                    addr_space="Shared")
                nc.gpsimd.dma_start(cc_din[:], st2[:, :, :])
                nc.gpsimd.collective_compute(
                    "AllReduce", Op.add,
                    replica_groups=[list(range(N_CORES))],
                    ins=[cc_din[:]], outs=[cc_dout[:]])
                s_t = sbuf.tile([P, 4], F32, name=f"s_{tagn}")
                nc.sync.dma_start(s_t[:], cc_dout[:, :])
                sm = s_t[:, 0:4:2]
                sq = s_t[:, 1:4:2]
                # u = 8*Sq - Sm^2  (= 64*var);  A = G8 * rsqrt(u + E64)
                t0 = sbuf.tile([P, NB], F32, name=f"t0_{tagn}")
                u = sbuf.tile([P, NB], F32, name=f"u_{tagn}")
                wv = sbuf.tile([P, NB], F32, name=f"w_{tagn}")
                r = sbuf.tile([P, NB], F32, name=f"r_{tagn}")
                av = sbuf.tile([P, NB], F32, name=f"av_{tagn}")
                t1 = sbuf.tile([P, NB], F32, name=f"t1_{tagn}")
                bv = sbuf.tile([P, NB], F32, name=f"bv_{tagn}")
                nc.vector.tensor_tensor(t0[:], sm, sm, Op.mult)
                nc.vector.scalar_tensor_tensor(u[:], sq, 8.0, t0[:],
                                               Op.mult, Op.subtract)
                nc.vector.tensor_tensor(wv[:], u[:], e64_ap, Op.add)
                nc.scalar.activation(t0[:], wv[:], F.Sqrt)
                nc.vector.reciprocal(r[:], t0[:])
                nc.vector.tensor_tensor(av[:], g8_ap, r[:], Op.mult)
                # B = beta - (Sm/8)*A
                nc.vector.tensor_tensor(t1[:], sm, av[:], Op.mult)
                nc.vector.scalar_tensor_tensor(bv[:], t1[:], -0.125, be_ap,
                                               Op.mult, Op.add)
                return av, bv

            a1v, b1v = stats_to_ab(st6_1, g8_1, e64_1, be1, "c1")

            # ================= post1 + conv2, software-pipelined ===========
            scr = [sbuf.tile([P, NB, HW], F32, name=f"scr{i}")
                   for i in range(3)]
            scrv = [t.rearrange("p k (r c) -> p k r c", c=W) for t in scr]
            scr2 = [sbuf.tile([P, NB, HW], BF16, name=f"sc2{i}")
                    for i in range(3)]

            # image 0 in two row chunks so conv2 starts early:
            # chunk A = rows 0..14 (enough for the hh=0 output half)
            for m in range(NB):
                nc.vector.affine_then_add(
                    scr[0][:, m, 0:CH0], y_sb[:, 0, m, 0:CH0],
                    x_sb[:, 0, m, 0:CH0],
                    scale=a1v[:, m:m + 1], bias=b1v[:, m:m + 1])
                # per-k sign so k=0 overlaps the k=1 affine
                nc.scalar.sign(xbv[0][:, m, 1:16, 1:29],
                               scrv[0][:, m, 0:15, :])
            for m in range(NB):
                nc.vector.affine_then_add(
                    scr[0][:, m, CH0:HW], y_sb[:, 0, m, CH0:HW],
                    x_sb[:, 0, m, CH0:HW],
                    scale=a1v[:, m:m + 1], bias=b1v[:, m:m + 1])
            nc.scalar.sign(xbv[0][:, :, 16:29, 1:29], scrv[0][:, :, 15:28, :])

            def post1(b):
                """A1*y1 + B1 + x -> scr; sign -> conv2 input. PReLU deferred."""
                s = scr[b % 3]
                sv = scrv[b % 3]
                for m in range(NB):
                    nc.vector.affine_then_add(
                        s[:, m, :], y_sb[:, b, m, :], x_sb[:, b, m, :],
                        scale=a1v[:, m:m + 1], bias=b1v[:, m:m + 1])
                nc.scalar.sign(xbv[b][:, :, 1:29, 1:29], sv[:, :, :, :])

            for b in range(BL):
                if b + 1 < BL:
                    post1(b + 1)
                conv_img(w2_sb, xh_sb[:, b], st6_2, b)  # y2 in bf16, over xh
                # deferred PReLU: res2 = prelu(scr[b]) in bf16
                nc.scalar.activation(
                    r2_sb[:, b].rearrange("p k i -> p (k i)"),
                    scr[b % 3].rearrange("p k i -> p (k i)"),
                    F.Prelu, bias=0.0, scale=1.0, alpha=a1_ap)

            a2v, b2v = stats_to_ab(st6_2, g8_2, e64_2, be2, "c2")

            # ================= post2 (tail), bf16 DVE + ACT prelu ==========
            o_dst = o_d.rearrange("b (k p) h w -> b p k (h w)", p=P)
            o_dst2 = o_d.rearrange("b (k p) h w -> p b k (h w)", p=P)
            order = list(range(BL))
            bufidx = {b: i % 3 for i, b in enumerate(order)}

            def post2_u(b):
                # bf16 u = (A2*y2 + B2) + res2: per-m tensor_scalar hits the
                # DVE 16-bit packing modes (affine_then_add can't), then one
                # full-image tensor_tensor add (on GpSimd for images 5-7).
                s = scr2[bufidx[b]]
                for m in range(NB):
                    nc.vector.tensor_scalar(
                        s[:, m, :], xh_sb[:, b, m, :],
                        a2v[:, m:m + 1], b2v[:, m:m + 1], Op.mult, Op.add)
                nc.vector.tensor_tensor(s[:, :, :], s[:, :, :], r2_sb[:, b],
                                        Op.add)

            def post2_prelu(b):
                s = scr2[bufidx[b]]
                # bf16 out (host upcasts): halves the final DMA drain
                nc.scalar.activation(
                    r2_sb[:, b].rearrange("p k i -> p (k i)"),
                    s.rearrange("p k i -> p (k i)"),
                    F.Prelu, bias=0.0, scale=1.0, alpha=a2_ap)
                # out DMAs carry ~1.5-2us fixed cost and serialize per
                # queue: spread over 3 queues, last image on the idle one
                eng = (nc.sync, nc.gpsimd, nc.sync, nc.gpsimd,
                       nc.sync, nc.gpsimd, nc.sync, nc.scalar)[b]
                eng.dma_start(o_dst[b], r2_sb[:, b])

            emitted = []
            for b in order:
                post2_u(b)
                # drain prelus whose u is ready (keeps chains tight)
                if len(emitted) >= 1:
                    post2_prelu(emitted.pop(0))
                emitted.append(b)
            while emitted:
                post2_prelu(emitted.pop(0))

    nc.compile()
    return nc


def _get_nc():
    if "nc" not in _CACHE:
        _CACHE["nc"] = _build()
    return _CACHE["nc"]


def _pack_w(w):
    wb = np.sign(np.asarray(w, np.float32))
    # [co, ci, kh, kw] -> [ki, tap, i, co_blk, co] with ci = i*128 + ki
    t = wb.reshape(NB, P, NB, P, 3, 3)
    t = np.transpose(t, (3, 4, 5, 2, 0, 1)).reshape(P, 9, NB, NB, P)
    return np.ascontiguousarray(t).astype(np.dtype(ml_dtypes.float8_e4m3))


def _pack_par(conv1_w, conv2_w, g1, be1, g2, be2, a1, a2):
    s1 = SCALE * np.mean(np.abs(np.asarray(conv1_w, np.float32)),
                         axis=(1, 2, 3), dtype=np.float32)
    s2 = SCALE * np.mean(np.abs(np.asarray(conv2_w, np.float32)),
                         axis=(1, 2, 3), dtype=np.float32)
    g1 = np.asarray(g1, np.float32)
    g2 = np.asarray(g2, np.float32)
    slots = [
        8.0 * g1,                                  # G8_1
        64.0 * BN_EPS / (s1 * s1),                 # E64_1
        np.asarray(be1, np.float32),
        8.0 * g2,                                  # G8_2
        64.0 * BN_EPS / (s2 * s2),                 # E64_2
        np.asarray(be2, np.float32),
        np.concatenate([np.full(P, np.float32(np.asarray(a1).reshape(())),
                                dtype=np.float32),
                        np.full(P, np.float32(np.asarray(a2).reshape(())),
                                dtype=np.float32)]),  # alpha slot [2*P]
    ]
    par = np.stack([np.asarray(v, np.float32).reshape(NB, P)
                    for v in slots])                # [NSLOT, NB, P]
    return np.ascontiguousarray(par.transpose(2, 0, 1))  # [P, NSLOT, NB]


def _prep(x, conv1_w, conv2_w, bn1_gamma, bn1_beta, bn2_gamma, bn2_beta,
          prelu1_a, prelu2_a):
    x = np.ascontiguousarray(np.asarray(x, np.float32))
    shared = {
        "w1": _pack_w(conv1_w), "w2": _pack_w(conv2_w),
        "par": _pack_par(conv1_w, conv2_w, bn1_gamma, bn1_beta,
                         bn2_gamma, bn2_beta, prelu1_a, prelu2_a),
    }
    xh = x.astype(BF16_NP)
    return [dict(shared, x=x[c * BL:(c + 1) * BL],
                 xh=xh[c * BL:(c + 1) * BL]) for c in range(N_CORES)]


def kernel(x, conv1_w, conv2_w, bn1_gamma, bn1_beta, bn2_gamma, bn2_beta,
           prelu1_a, prelu2_a):
    nc = _get_nc()
    in_maps = _prep(x, conv1_w, conv2_w, bn1_gamma, bn1_beta,
                    bn2_gamma, bn2_beta, prelu1_a, prelu2_a)
    res = bass_utils.run_bass_kernel_spmd(nc, in_maps,
                                          core_ids=list(range(N_CORES)))
    out = np.concatenate([res.results[c]["o"] for c in range(N_CORES)],
                         axis=0)
    return out.astype(np.float32)

